# Initial kernel scaffold
#
"""Trainium2 Bass/Tile kernel for nn_AttnBlock_29712583753795.

Per sample (B=16, C=512, H=W=64, n=4096):
    xn  = groupnorm(x; 16 groups, w1, b1)
    kv  = kv_w @ xn + kv_b                  (1x1 conv -> [2C, n])
    k, v = split(kv)
    q   = softmax_c(k) * C^-0.5
    k   = softmax_n(k)
    ctx = k @ v.T                           [C, C]
    o2  = ctx.T @ q                         [C, n]
    out = out_w @ gelu(groupnorm(o2; w2, b2)) + out_b
    return xn + out

Sharding: pure data-parallel over batch. 2 samples per NeuronCore, 8 cores.

Key algebraic folds (avoid all transposes of the big [C, n] tensors):
  * softmax_n(k) is invariant to the k-bias (constant along n) -> the kv-pass
    that feeds the context matmul needs NO bias at all.
  * context row-normalization (1/R[d]) and the v-bias fold into cheap
    per-partition / small-tile ops on the [C, C] context matrix:
        ctx_final = ctx_raw / R[d] + vb[e]
  * the q-softmax normalizer S[n] rides the attention matmul as an extra
    ones-vector matmul; 1/S is broadcast across partitions with a K=1 matmul.
  * k is computed twice (once as k^T [n,C] for the context contraction over n,
    once as k [C,n] for the attention contraction over d) - cheaper than
    materializing an 8MB transpose.
All big matmuls run as float32r (full fp32 data, relaxed PE mode, 1 cyc/row).
"""

import sys

for _p in ("/opt/trn_rl_repo",):
    if _p not in sys.path:
        sys.path.insert(0, _p)

import numpy as np

import concourse.bass as bass
import concourse.tile as tile
from concourse import bacc, mybir
from concourse.bass_utils import run_bass_kernel_spmd

F32 = mybir.dt.float32
F32R = mybir.dt.float32r
AX = mybir.AxisListType
OP = mybir.AluOpType
AF = mybir.ActivationFunctionType

N_CORES = 8
B, C, H, W = 16, 512, 64, 64
N = H * W                      # 4096 spatial
BPC = B // N_CORES             # 2 samples per core
P = 128                        # partitions
CT = C // P                    # 4 channel tiles
NT = N // P                    # 32 n-tiles (phase 1)
NCH = N // 512                 # 8 n-chunks of 512 (phases 2/3)
GROUPS = 16
GSIZE = C // GROUPS            # 32 channels per group
GN_COUNT = float(GSIZE * N)    # 131072 elements per group
EPS = 1e-5
QINV = float(np.sqrt(float(C)))  # 1/q_scale


def _r(ap):
    return ap.bitcast(F32R)


def build_program(gelu: bool = True):
    """Build the per-core Bass program (identical on all 8 cores)."""
    nc = bacc.Bacc("TRN2", target_bir_lowering=False, debug=False,
                   num_devices=N_CORES)

    x_d = nc.dram_tensor("x", [BPC * C, N], F32, kind="ExternalInput").ap()
    kvw_d = nc.dram_tensor("kvwT", [C, 2 * C], F32, kind="ExternalInput").ap()
    outw_d = nc.dram_tensor("outwT", [C, C], F32, kind="ExternalInput").ap()
    prm_d = nc.dram_tensor("prm", [6, CT, P], F32, kind="ExternalInput").ap()
    vb_d = nc.dram_tensor("vb", [1, C], F32, kind="ExternalInput").ap()
    gm_d = nc.dram_tensor("gmat", [P, 4], F32, kind="ExternalInput").ap()
    gmT_d = nc.dram_tensor("gmatT", [4, P], F32, kind="ExternalInput").ap()
    out_d = nc.dram_tensor("out", [BPC * C, N], F32, kind="ExternalOutput").ap()

    gelu_f = AF.Gelu if gelu else AF.Identity

    with tile.TileContext(nc) as tc:
        from contextlib import ExitStack
        with ExitStack() as ctx:
            E = ctx.enter_context
            const = E(tc.tile_pool(name="const", bufs=1))
            x_pool = E(tc.tile_pool(name="x", bufs=5))
            out2_pool = E(tc.tile_pool(name="out2", bufs=4))
            ctxsb_pool = E(tc.tile_pool(name="ctxsb", bufs=4))
            ekt_pool = E(tc.tile_pool(name="ekt", bufs=3))
            vt_pool = E(tc.tile_pool(name="vt", bufs=3))
            ek2_pool = E(tc.tile_pool(name="ek2", bufs=3))
            g_pool = E(tc.tile_pool(name="g", bufs=3))
            bcs_pool = E(tc.tile_pool(name="bcs", bufs=2))
            outsb_pool = E(tc.tile_pool(name="outsb", bufs=3))
            dump_pool = E(tc.tile_pool(name="dump", bufs=2))
            stat_pool = E(tc.tile_pool(name="stat", bufs=8))
            ab_pool = E(tc.tile_pool(name="ab", bufs=10))
            small_pool = E(tc.tile_pool(name="small", bufs=6))
            dram_pool = E(tc.tile_pool(name="drsc", bufs=2, space="DRAM"))

            kv_ps = E(tc.tile_pool(name="kv_ps", bufs=3, space="PSUM"))
            ctx_ps_pool = E(tc.tile_pool(name="ctx_ps", bufs=4, space="PSUM"))
            r_ps_pool = E(tc.tile_pool(name="r_ps", bufs=1, space="PSUM"))
            k2_ps = E(tc.tile_pool(name="k2_ps", bufs=2, space="PSUM"))
            o2_ps_pool = E(tc.tile_pool(name="o2_ps", bufs=4, space="PSUM"))
            s_ps_pool = E(tc.tile_pool(name="s_ps", bufs=1, space="PSUM"))
            bc_ps_pool = E(tc.tile_pool(name="bc_ps", bufs=1, space="PSUM"))
            o3_ps_pool = E(tc.tile_pool(name="o3_ps", bufs=4, space="PSUM"))
            gn_ps = E(tc.tile_pool(name="gn_ps", bufs=2, space="PSUM"))

            # ---------------- constants ----------------
            kvw_sb = const.tile([P, CT * 2 * C], F32)    # [128, 8192]
            for ct in range(CT):
                nc.sync.dma_start(kvw_sb[:, ct * 2 * C:(ct + 1) * 2 * C],
                                  kvw_d[ct * P:(ct + 1) * P, :])
            outw_sb = const.tile([P, CT * C], F32)       # [128, 2048]
            for et in range(CT):
                nc.sync.dma_start(outw_sb[:, et * C:(et + 1) * C],
                                  outw_d[et * P:(et + 1) * P, :])
            # per-channel params, one [128, CT] tile each: w1,b1,kb,w2,b2,ob
            pcols = []
            for idx in range(6):
                t = const.tile([P, CT], F32)
                nc.sync.dma_start(t, prm_d[idx].rearrange("t p -> p t"))
                pcols.append(t)
            w1c, b1c, kbc, w2c, b2c, obc = pcols
            gm = const.tile([P, 4], F32)
            nc.sync.dma_start(gm, gm_d)
            gmT = const.tile([4, P], F32)
            nc.sync.dma_start(gmT, gmT_d)
            vb_row = const.tile([1, C], F32)
            nc.sync.dma_start(vb_row, vb_d)
            ones_col = const.tile([P, 1], F32)
            nc.vector.memset(ones_col, 1.0)
            qinv_col = const.tile([P, 1], F32)
            nc.vector.memset(qinv_col, QINV)
            ones_row = const.tile([1, P], F32)
            nc.vector.memset(ones_row, 1.0)
            # vb broadcast to all partitions via K=1 matmul
            vb_bcast = const.tile([P, C], F32)
            bps0 = bc_ps_pool.tile([P, C], F32)
            nc.tensor.matmul(bps0, _r(ones_row), _r(vb_row), start=True, stop=True)
            nc.scalar.copy(vb_bcast, bps0)

            def gn_params(stats2, wcol, bcol):
                """stats2: [128,2] SBUF (sum, sumsq) per channel ->
                ab [128,2] tile: A = w*rstd, B = b - mu*A."""
                gps = gn_ps.tile([4, 2], F32)
                nc.tensor.matmul(gps, gm, stats2, start=True, stop=True)
                gmn = stat_pool.tile([4, 2], F32)
                nc.vector.tensor_scalar_mul(gmn, gps, 1.0 / GN_COUNT)
                musq = stat_pool.tile([4, 1], F32)
                nc.vector.tensor_mul(musq, gmn[:, 0:1], gmn[:, 0:1])
                murstd = stat_pool.tile([4, 2], F32)
                varv = stat_pool.tile([4, 1], F32)
                nc.vector.tensor_sub(varv, gmn[:, 1:2], musq)
                stdv = stat_pool.tile([4, 1], F32)
                nc.scalar.activation(stdv, varv, AF.Sqrt, bias=EPS)
                nc.vector.reciprocal(murstd[:, 1:2], stdv)
                nc.vector.tensor_copy(murstd[:, 0:1], gmn[:, 0:1])
                cps = gn_ps.tile([P, 2], F32)
                nc.tensor.matmul(cps, gmT, murstd, start=True, stop=True)
                ab = ab_pool.tile([P, 2], F32)
                nc.vector.tensor_mul(ab[:, 0:1], wcol, cps[:, 1:2])
                tmpm = stat_pool.tile([P, 1], F32)
                nc.vector.tensor_mul(tmpm, cps[:, 0:1], ab[:, 0:1])
                nc.vector.tensor_sub(ab[:, 1:2], bcol, tmpm)
                return ab

            for s in range(BPC):
                row0 = s * C
                # ================= GroupNorm 1 (streaming per ctile) ========
                xn = []
                for ct in range(CT):
                    xt = x_pool.tile([P, N], F32)
                    nc.sync.dma_start(xt, x_d[row0 + ct * P: row0 + (ct + 1) * P, :])
                    sm8 = stat_pool.tile([P, 8], F32)
                    sq8 = stat_pool.tile([P, 8], F32)
                    for j in range(NCH):
                        sl = xt[:, j * 512:(j + 1) * 512]
                        dmp = dump_pool.tile([P, 512], F32)
                        nc.scalar.activation(dmp, sl, AF.Square,
                                             accum_out=sq8[:, j:j + 1])
                        nc.vector.reduce_sum(sm8[:, j:j + 1], sl, axis=AX.X)
                    st2 = stat_pool.tile([P, 2], F32)
                    nc.vector.reduce_sum(st2[:, 0:1], sm8, axis=AX.X)
                    nc.vector.reduce_sum(st2[:, 1:2], sq8, axis=AX.X)
                    ab1 = gn_params(st2, w1c[:, ct:ct + 1], b1c[:, ct:ct + 1])
                    # xn in-place: x*A + B
                    nc.vector.tensor_scalar(xt, xt, ab1[:, 0:1], ab1[:, 1:2],
                                            op0=OP.mult, op1=OP.add)
                    xn.append(xt)

                # ================= Phase 1: kv^T pass + context =============
                ctx_acc = [ctx_ps_pool.tile([P, C], F32) for _ in range(CT)]
                r_row = r_ps_pool.tile([1, C], F32)

                def emit_ctx(ekt, vt, nt):
                    nc.tensor.matmul(r_row, _r(ones_col), _r(ekt),
                                     start=(nt == 0), stop=(nt == NT - 1))
                    for dt in range(CT):
                        nc.tensor.matmul(ctx_acc[dt],
                                         _r(ekt[:, dt * P:(dt + 1) * P]), _r(vt),
                                         start=(nt == 0), stop=(nt == NT - 1))

                prev = None
                for nt in range(NT):
                    kps = kv_ps.tile([P, 512], F32)
                    for ct in range(CT):
                        nc.tensor.matmul(
                            kps, _r(xn[ct][:, nt * P:(nt + 1) * P]),
                            _r(kvw_sb[:, ct * 2 * C: ct * 2 * C + 512]),
                            start=(ct == 0), stop=(ct == CT - 1))
                    vps = kv_ps.tile([P, 512], F32)
                    for ct in range(CT):
                        nc.tensor.matmul(
                            vps, _r(xn[ct][:, nt * P:(nt + 1) * P]),
                            _r(kvw_sb[:, ct * 2 * C + 512: (ct + 1) * 2 * C]),
                            start=(ct == 0), stop=(ct == CT - 1))
                    ekt = ekt_pool.tile([P, 512], F32)
                    nc.scalar.activation(ekt, kps, AF.Exp)  # k-bias cancels
                    vt = vt_pool.tile([P, 512], F32)
                    nc.vector.tensor_copy(vt, vps)          # v-bias folded later
                    if prev is not None:
                        emit_ctx(*prev)
                    prev = (ekt, vt, nt)
                emit_ctx(*prev)

                # R: [1,512] row -> per-partition columns via DRAM bounce
                r_sb = small_pool.tile([1, C], F32)
                nc.scalar.copy(r_sb, r_row)
                rb = dram_pool.tile([1, C], F32)
                nc.sync.dma_start(rb, r_sb)
                rcol = small_pool.tile([P, CT], F32)
                nc.sync.dma_start(rcol, rb.rearrange("a (t p) -> (a p) t", p=P))
                rcp = small_pool.tile([P, CT], F32)
                nc.vector.reciprocal(rcp, rcol)
                ctx_sb = []
                for dt in range(CT):
                    t = ctxsb_pool.tile([P, C], F32)
                    nc.vector.tensor_scalar_mul(t, ctx_acc[dt], rcp[:, dt:dt + 1])
                    nc.vector.tensor_add(t, t, vb_bcast)
                    ctx_sb.append(t)

                # ================= Phase 2: k pass + attention out ==========
                out2 = [out2_pool.tile([P, N], F32) for _ in range(CT)]
                s2_8 = [stat_pool.tile([P, 8], F32) for _ in range(CT)]
                q2_8 = [stat_pool.tile([P, 8], F32) for _ in range(CT)]
                o2ps = {}
                sps = {}

                def emit_attn(j, dt, ek2):
                    nc.tensor.matmul(sps[j], _r(qinv_col), _r(ek2),
                                     start=(dt == 0), stop=(dt == CT - 1))
                    for et in range(CT):
                        nc.tensor.matmul(o2ps[j][et],
                                         _r(ctx_sb[dt][:, et * P:(et + 1) * P]),
                                         _r(ek2),
                                         start=(dt == 0), stop=(dt == CT - 1))
                    if dt == CT - 1:
                        # drain chunk j: 1/S broadcast, scale, GN2 stats
                        rcs = small_pool.tile([1, 512], F32)
                        nc.vector.reciprocal(rcs, sps[j][0:1, :])
                        bps = bc_ps_pool.tile([P, 512], F32)
                        nc.tensor.matmul(bps, _r(ones_row), _r(rcs),
                                         start=True, stop=True)
                        bcs = bcs_pool.tile([P, 512], F32)
                        nc.scalar.copy(bcs, bps)
                        for et in range(CT):
                            sl = out2[et][:, j * 512:(j + 1) * 512]
                            nc.vector.tensor_mul(sl, o2ps[j][et], bcs)
                            dmp = dump_pool.tile([P, 512], F32)
                            nc.scalar.activation(dmp, sl, AF.Square,
                                                 accum_out=q2_8[et][:, j:j + 1])
                            nc.vector.reduce_sum(s2_8[et][:, j:j + 1], sl,
                                                 axis=AX.X)
                        del o2ps[j], sps[j]

                prev2 = None
                for j in range(NCH):
                    o2ps[j] = [o2_ps_pool.tile([P, 512], F32) for _ in range(CT)]
                    sps[j] = s_ps_pool.tile([1, 512], F32)
                    for dt in range(CT):
                        k2 = k2_ps.tile([P, 512], F32)
                        for ct in range(CT):
                            nc.tensor.matmul(
                                k2,
                                _r(kvw_sb[:, ct * 2 * C + dt * P:
                                          ct * 2 * C + (dt + 1) * P]),
                                _r(xn[ct][:, j * 512:(j + 1) * 512]),
                                start=(ct == 0), stop=(ct == CT - 1))
                        ek2 = ek2_pool.tile([P, 512], F32)
                        nc.scalar.activation(ek2, k2, AF.Exp,
                                             bias=kbc[:, dt:dt + 1])
                        if prev2 is not None:
                            emit_attn(*prev2)
                        prev2 = (j, dt, ek2)
                emit_attn(*prev2)

                # ================= GroupNorm 2 params =======================
                ab2 = []
                for et in range(CT):
                    st2 = stat_pool.tile([P, 2], F32)
                    nc.vector.reduce_sum(st2[:, 0:1], s2_8[et], axis=AX.X)
                    nc.vector.reduce_sum(st2[:, 1:2], q2_8[et], axis=AX.X)
                    ab2.append(gn_params(st2, w2c[:, et:et + 1], b2c[:, et:et + 1]))

                # ================= Phase 3: gelu + proj + residual ==========
                def emit_proj(j, gts):
                    for ot in range(CT):
                        o3 = o3_ps_pool.tile([P, 512], F32)
                        for et in range(CT):
                            nc.tensor.matmul(
                                o3,
                                _r(outw_sb[:, et * C + ot * P: et * C + (ot + 1) * P]),
                                _r(gts[et]),
                                start=(et == 0), stop=(et == CT - 1))
                        ob_sb = outsb_pool.tile([P, 512], F32)
                        nc.scalar.activation(ob_sb, o3, AF.Identity,
                                             bias=obc[:, ot:ot + 1])
                        nc.vector.tensor_add(ob_sb, ob_sb,
                                             xn[ot][:, j * 512:(j + 1) * 512])
                        nc.sync.dma_start(
                            out_d[row0 + ot * P: row0 + (ot + 1) * P,
                                  j * 512:(j + 1) * 512], ob_sb)

                prev3 = None
                for j in range(NCH):
                    gts = []
                    for et in range(CT):
                        g = g_pool.tile([P, 512], F32)
                        nc.scalar.activation(g, out2[et][:, j * 512:(j + 1) * 512],
                                             gelu_f, bias=ab2[et][:, 1:2],
                                             scale=ab2[et][:, 0:1])
                        gts.append(g)
                    if prev3 is not None:
                        emit_proj(*prev3)
                    prev3 = (j, gts)
                emit_proj(*prev3)

    nc.compile()
    return nc


def prep_inputs(inputs):
    """Host-side prep: shard x over batch, pre-transpose/pack weights."""
    x = np.ascontiguousarray(np.asarray(inputs["x"], dtype=np.float32))
    kv_w = np.asarray(inputs["kv_w"], dtype=np.float32)
    kv_b = np.asarray(inputs["kv_b"], dtype=np.float32)
    out_w = np.asarray(inputs["out_w"], dtype=np.float32)
    out_b = np.asarray(inputs["out_b"], dtype=np.float32)
    w1 = np.asarray(inputs["norm1_w"], dtype=np.float32)
    b1 = np.asarray(inputs["norm1_b"], dtype=np.float32)
    w2 = np.asarray(inputs["norm2_w"], dtype=np.float32)
    b2 = np.asarray(inputs["norm2_b"], dtype=np.float32)

    kvwT = np.ascontiguousarray(kv_w.T)                 # [C, 2C]
    outwT = np.ascontiguousarray(out_w.T)               # [C, C]
    kb = kv_b[:C]
    vb = np.ascontiguousarray(kv_b[C:]).reshape(1, C)
    prm = np.stack([w1, b1, kb, w2, b2, out_b]).reshape(6, CT, P)
    prm = np.ascontiguousarray(prm)
    gmat = np.zeros((P, 4), np.float32)
    for p in range(P):
        gmat[p, p // GSIZE] = 1.0
    gmatT = np.ascontiguousarray(gmat.T)

    xs = x.reshape(B, C, N)
    in_maps = []
    for i in range(N_CORES):
        shard = np.ascontiguousarray(
            xs[i * BPC:(i + 1) * BPC].reshape(BPC * C, N))
        in_maps.append({
            "x": shard, "kvwT": kvwT, "outwT": outwT, "prm": prm,
            "vb": vb, "gmat": gmat, "gmatT": gmatT,
        })
    return in_maps


_NC_CACHE = {}


def get_program(gelu: bool = True):
    key = bool(gelu)
    if key not in _NC_CACHE:
        _NC_CACHE[key] = build_program(gelu=key)
    return _NC_CACHE[key]


def run(inputs, trace: bool = False, gelu: bool = True):
    """Run on 8 cores; returns (full_output [16,512,64,64], BassKernelResults)."""
    nc = get_program(gelu=gelu)
    in_maps = prep_inputs(inputs)
    res = run_bass_kernel_spmd(nc, in_maps, core_ids=list(range(N_CORES)),
                               trace=trace)
    full = np.empty((B, C, N), np.float32)
    for i in range(N_CORES):
        full[i * BPC:(i + 1) * BPC] = res.results[i]["out"].reshape(BPC, C, N)
    return full.reshape(B, C, H, W), res


def kernel(**inputs) -> np.ndarray:
    out, _ = run(inputs, trace=False, gelu=True)
    return out


# revision 15
# speedup vs baseline: 2.4295x; 2.4295x over previous
"""Trainium2 Bass/Tile kernel for nn_AttnBlock_29712583753795.

Per sample (B=16, C=512, H=W=64, n=4096):
    xn  = groupnorm(x; 16 groups, w1, b1)
    kv  = kv_w @ xn + kv_b                  (1x1 conv -> [2C, n])
    k, v = split(kv)
    q   = softmax_c(k) * C^-0.5
    k   = softmax_n(k)
    ctx = k @ v.T                           [C, C]
    o2  = ctx.T @ q                         [C, n]
    out = out_w @ gelu(groupnorm(o2; w2, b2)) + out_b
    return xn + out

Sharding: pure data-parallel over batch. 2 samples per NeuronCore, 8 cores.

Key algebraic folds (avoid all transposes of the big [C, n] tensors):
  * softmax_n(k) is invariant to the k-bias (constant along n) -> the kv-pass
    that feeds the context matmul needs NO bias at all.
  * context row-normalization (1/R[d]) and the v-bias fold into cheap
    per-partition / small-tile ops on the [C, C] context matrix:
        ctx_final = ctx_raw / R[d] + vb[e]
  * the q-softmax normalizer S[n] rides the attention matmul as an extra
    ones-vector matmul; 1/S is broadcast across partitions with a K=1 matmul.
  * k is computed twice (once as k^T [n,C] for the context contraction over n,
    once as k [C,n] for the attention contraction over d) - cheaper than
    materializing an 8MB transpose.
All big matmuls run as float32r (full fp32 data, relaxed PE mode, 1 cyc/row).
"""

import sys

for _p in ("/opt/trn_rl_repo",):
    if _p not in sys.path:
        sys.path.insert(0, _p)

import numpy as np

import concourse.bass as bass
import concourse.tile as tile
from concourse import bacc, mybir
from concourse.bass_utils import run_bass_kernel_spmd

F32 = mybir.dt.float32
F32R = mybir.dt.float32r
AX = mybir.AxisListType
OP = mybir.AluOpType
AF = mybir.ActivationFunctionType

N_CORES = 8
B, C, H, W = 16, 512, 64, 64
N = H * W                      # 4096 spatial
BPC = B // N_CORES             # 2 samples per core
P = 128                        # partitions
CT = C // P                    # 4 channel tiles
NT = N // P                    # 32 n-tiles (phase 1)
NCH = N // 512                 # 8 n-chunks of 512 (phases 2/3)
GROUPS = 16
GSIZE = C // GROUPS            # 32 channels per group
GN_COUNT = float(GSIZE * N)    # 131072 elements per group
EPS = 1e-5
QINV = float(np.sqrt(float(C)))  # 1/q_scale


def _r(ap):
    return ap.bitcast(F32R)


def build_program(gelu: bool = True, reps: int = 1):
    """Build the per-core Bass program (identical on all 8 cores)."""
    nc = bacc.Bacc("TRN2", target_bir_lowering=False, debug=False,
                   num_devices=N_CORES)

    x_d = nc.dram_tensor("x", [BPC * C, N], F32, kind="ExternalInput").ap()
    kvw_d = nc.dram_tensor("kvwT", [C, 2 * C], F32, kind="ExternalInput").ap()
    outw_d = nc.dram_tensor("outwT", [C, C], F32, kind="ExternalInput").ap()
    prm_d = nc.dram_tensor("prm", [6, CT, P], F32, kind="ExternalInput").ap()
    vb_d = nc.dram_tensor("vb", [1, C], F32, kind="ExternalInput").ap()
    gm_d = nc.dram_tensor("gmat", [P, 4], F32, kind="ExternalInput").ap()
    gmT_d = nc.dram_tensor("gmatT", [4, P], F32, kind="ExternalInput").ap()
    out_d = nc.dram_tensor("out", [BPC * C, N], F32, kind="ExternalOutput").ap()

    gelu_f = AF.Gelu if gelu else AF.Identity

    with tile.TileContext(nc) as tc:
        from contextlib import ExitStack
        with ExitStack() as ctx:
            E = ctx.enter_context
            const = E(tc.tile_pool(name="const", bufs=1))
            x_pool = E(tc.tile_pool(name="x", bufs=4))
            xs_pool = E(tc.tile_pool(name="xs", bufs=3))
            out2_pool = E(tc.tile_pool(name="out2", bufs=4))
            ctxsb_pool = E(tc.tile_pool(name="ctxsb", bufs=4))
            ekt_pool = E(tc.tile_pool(name="ekt", bufs=2))
            vt_pool = E(tc.tile_pool(name="vt", bufs=2))
            ek2_pool = E(tc.tile_pool(name="ek2", bufs=2))
            g_pool = E(tc.tile_pool(name="g", bufs=5))
            bcs_pool = E(tc.tile_pool(name="bcs", bufs=1))
            outsb_pool = E(tc.tile_pool(name="outsb", bufs=2))
            dump_pool = E(tc.tile_pool(name="dump", bufs=1))
            stat_pool = E(tc.tile_pool(name="stat", bufs=4))
            ab_pool = E(tc.tile_pool(name="ab", bufs=10))
            small_pool = E(tc.tile_pool(name="small", bufs=1))
            dram_pool = E(tc.tile_pool(name="drsc", bufs=2, space="DRAM"))

            # PSUM: 8 banks total, statically reserved -> share 3 pools
            # across phases via common tags (each slot = one [128,512] bank).
            quad_ps = E(tc.tile_pool(name="quad_ps", bufs=4, space="PSUM"))
            tri_ps = E(tc.tile_pool(name="tri_ps", bufs=3, space="PSUM"))
            row_ps = E(tc.tile_pool(name="row_ps", bufs=1, space="PSUM"))

            # ---------------- constants ----------------
            # f32r matmul operands must be written by a rounding instruction:
            # DMA weights into staging, DVE-copy into the const tiles as f32r.
            def stage_round(dst_slice, src_slice, rows=P):
                stg = dump_pool.tile([P, 512], F32, name="stg", tag="dump")
                nc.sync.dma_start(stg[:rows, :src_slice.shape[-1]], src_slice)
                nc.vector.tensor_copy(_r(dst_slice),
                                      stg[:rows, :src_slice.shape[-1]])

            kvw_sb = const.tile([P, CT * 2 * C], F32)    # [128, 4096]
            for ct in range(CT):
                for h in range(2):
                    stage_round(
                        kvw_sb[:, ct * 2 * C + h * 512: ct * 2 * C + (h + 1) * 512],
                        kvw_d[ct * P:(ct + 1) * P, h * 512:(h + 1) * 512])
            outw_sb = const.tile([P, CT * C], F32)       # [128, 2048]
            for et in range(CT):
                stage_round(outw_sb[:, et * C:(et + 1) * C],
                            outw_d[et * P:(et + 1) * P, :])
            # per-channel params, one [128, CT] tile each: w1,b1,kb,w2,b2,ob
            pcols = []
            for idx in range(6):
                t = const.tile([P, CT], F32, name=f"prm{idx}", tag=f"prm{idx}")
                nc.sync.dma_start(t, prm_d[idx].rearrange("t p -> p t"))
                pcols.append(t)
            w1c, b1c, kbc, w2c, b2c, obc = pcols
            gm = const.tile([P, 4], F32)
            nc.sync.dma_start(gm, gm_d)
            gmT = const.tile([4, P], F32)
            nc.sync.dma_start(gmT, gmT_d)
            vb_row = const.tile([1, C], F32)
            stage_round(vb_row, vb_d, rows=1)
            ones_col = const.tile([P, 1], F32)
            qinv_col = const.tile([P, 1], F32)
            ones_row = const.tile([1, P], F32)
            mset = dump_pool.tile([P, 512], F32, name="mset", tag="dump")
            nc.vector.memset(mset[:, 0:P], 1.0)
            nc.vector.tensor_copy(_r(ones_col), mset[:, 0:1])
            nc.vector.tensor_copy(_r(ones_row), mset[0:1, 0:P])
            nc.vector.memset(mset[:, 1:2], QINV)
            nc.vector.tensor_copy(_r(qinv_col), mset[:, 1:2])
            # vb broadcast to all partitions via K=1 matmul
            vb_bcast = const.tile([P, C], F32)
            bps0 = tri_ps.tile([P, C], F32, name="bps0", tag="tri")
            nc.tensor.matmul(bps0, _r(ones_row), _r(vb_row), start=True, stop=True)
            nc.scalar.copy(vb_bcast, bps0)

            def gn_params(stats2, wcol, bcol):
                """stats2: [128,2] SBUF (sum, sumsq) per channel ->
                ab [128,2] tile: A = w*rstd, B = b - mu*A."""
                gps = tri_ps.tile([4, 2], F32, name="gps", tag="tri")
                nc.tensor.matmul(gps, gm, stats2, start=True, stop=True)
                gmn = stat_pool.tile([4, 2], F32)
                nc.vector.tensor_scalar_mul(gmn, gps, 1.0 / GN_COUNT)
                musq = stat_pool.tile([4, 1], F32)
                nc.vector.tensor_mul(musq, gmn[:, 0:1], gmn[:, 0:1])
                murstd = stat_pool.tile([4, 2], F32)
                varv = stat_pool.tile([4, 1], F32)
                nc.vector.tensor_sub(varv, gmn[:, 1:2], musq)
                nc.vector.tensor_scalar_add(varv, varv, EPS)
                stdv = stat_pool.tile([4, 1], F32)
                nc.scalar.activation(stdv, varv, AF.Sqrt)
                nc.vector.reciprocal(murstd[:, 1:2], stdv)
                nc.vector.tensor_copy(murstd[:, 0:1], gmn[:, 0:1])
                cps = tri_ps.tile([P, 2], F32, name="cps", tag="tri")
                nc.tensor.matmul(cps, gmT, murstd, start=True, stop=True)
                ab = ab_pool.tile([P, 2], F32)
                nc.vector.tensor_mul(ab[:, 0:1], wcol, cps[:, 1:2])
                tmpm = stat_pool.tile([P, 1], F32)
                nc.vector.tensor_mul(tmpm, cps[:, 0:1], ab[:, 0:1])
                nc.vector.tensor_sub(ab[:, 1:2], bcol, tmpm)
                return ab

            for s in [s for _ in range(reps) for s in range(BPC)]:
                row0 = s * C
                # ============ GroupNorm 1 (two streaming passes over x) =====
                # Pass 1: chunked stats; pass 2: re-DMA x, apply affine into a
                # clean xn tile (f32r writes only, as the f32r matmuls need).
                xn = []
                for ct in range(CT):
                    rows = slice(row0 + ct * P, row0 + (ct + 1) * P)
                    sm8 = stat_pool.tile([P, 8], F32)
                    sq8 = stat_pool.tile([P, 8], F32)
                    for j in range(NCH):
                        xc = xs_pool.tile([P, 512], F32, name="xc", tag="xc")
                        nc.sync.dma_start(xc, x_d[rows, j * 512:(j + 1) * 512])
                        dmp = dump_pool.tile([P, 512], F32)
                        nc.scalar.activation(dmp, xc, AF.Square,
                                             accum_out=sq8[:, j:j + 1])
                        nc.vector.reduce_sum(sm8[:, j:j + 1], xc, axis=AX.X)
                    st2 = stat_pool.tile([P, 2], F32)
                    nc.vector.reduce_sum(st2[:, 0:1], sm8, axis=AX.X)
                    nc.vector.reduce_sum(st2[:, 1:2], sq8, axis=AX.X)
                    ab1 = gn_params(st2, w1c[:, ct:ct + 1], b1c[:, ct:ct + 1])
                    xnt = x_pool.tile([P, N], F32, name="xnt", tag="xnt")
                    for j in range(NCH):
                        xc = xs_pool.tile([P, 512], F32, name="xc", tag="xc")
                        nc.sync.dma_start(xc, x_d[rows, j * 512:(j + 1) * 512])
                        nc.vector.tensor_scalar(_r(xnt[:, j * 512:(j + 1) * 512]),
                                                xc, ab1[:, 0:1], ab1[:, 1:2],
                                                op0=OP.mult, op1=OP.add)
                    xn.append(xnt)

                # ================= Phase 1: kv^T pass + context =============
                ctx_acc = [quad_ps.tile([P, C], F32, name="ctx_acc", tag="quad") for _ in range(CT)]
                r_row = row_ps.tile([1, C], F32, name="r_row", tag="row")

                def emit_ctx(ekt, vt, nt):
                    nc.tensor.matmul(r_row, _r(ones_col), _r(ekt),
                                     start=(nt == 0), stop=(nt == NT - 1))
                    for dt in range(CT):
                        nc.tensor.matmul(ctx_acc[dt],
                                         _r(ekt[:, dt * P:(dt + 1) * P]), _r(vt),
                                         start=(nt == 0), stop=(nt == NT - 1))

                prev = None
                for nt in range(NT):
                    kps = tri_ps.tile([P, 512], F32, name="kps", tag="tri")
                    for ct in range(CT):
                        nc.tensor.matmul(
                            kps, _r(xn[ct][:, nt * P:(nt + 1) * P]),
                            _r(kvw_sb[:, ct * 2 * C: ct * 2 * C + 512]),
                            start=(ct == 0), stop=(ct == CT - 1))
                    vps = tri_ps.tile([P, 512], F32, name="vps", tag="tri")
                    for ct in range(CT):
                        nc.tensor.matmul(
                            vps, _r(xn[ct][:, nt * P:(nt + 1) * P]),
                            _r(kvw_sb[:, ct * 2 * C + 512: (ct + 1) * 2 * C]),
                            start=(ct == 0), stop=(ct == CT - 1))
                    ekt = ekt_pool.tile([P, 512], F32)
                    nc.scalar.activation(_r(ekt), kps, AF.Exp)  # k-bias cancels
                    vt = vt_pool.tile([P, 512], F32)
                    nc.vector.tensor_copy(_r(vt), vps)      # v-bias folded later
                    if prev is not None:
                        emit_ctx(*prev)
                    prev = (ekt, vt, nt)
                emit_ctx(*prev)

                # R: [1,512] row -> per-partition columns via DRAM bounce
                r_sb = small_pool.tile([1, C], F32, name="r_sb", tag="rcs")
                nc.scalar.copy(r_sb, r_row)
                rb = dram_pool.tile([1, C], F32)
                nc.sync.dma_start(rb, r_sb)
                rcol = small_pool.tile([P, CT], F32)
                nc.sync.dma_start(rcol, rb.rearrange("a (t p) -> (a p) t", p=P))
                rcp = small_pool.tile([P, CT], F32)
                nc.vector.reciprocal(rcp, rcol)
                ctx_sb = []
                for dt in range(CT):
                    t = ctxsb_pool.tile([P, C], F32, name="ctx_sb", tag="ctx_sb")
                    ctmp = dump_pool.tile([P, 512], F32, name="ctmp", tag="dump")
                    nc.vector.tensor_scalar_mul(ctmp, ctx_acc[dt], rcp[:, dt:dt + 1])
                    nc.vector.tensor_add(_r(t), ctmp, vb_bcast)
                    ctx_sb.append(t)

                # ================= Phase 2: k pass + attention out ==========
                out2 = [out2_pool.tile([P, N], F32, name="out2", tag="out2") for _ in range(CT)]
                s2_8 = [stat_pool.tile([P, 8], F32, name="s2_8", tag="s2_8") for _ in range(CT)]
                q2_8 = [stat_pool.tile([P, 8], F32, name="q2_8", tag="q2_8") for _ in range(CT)]
                o2ps = {}
                sps = {}

                def emit_attn(j, dt, ek2):
                    nc.tensor.matmul(sps[j], _r(qinv_col), _r(ek2),
                                     start=(dt == 0), stop=(dt == CT - 1))
                    for et in range(CT):
                        nc.tensor.matmul(o2ps[j][et],
                                         _r(ctx_sb[dt][:, et * P:(et + 1) * P]),
                                         _r(ek2),
                                         start=(dt == 0), stop=(dt == CT - 1))
                    if dt == CT - 1:
                        # drain chunk j: 1/S broadcast, scale, GN2 stats
                        rcs = small_pool.tile([1, 512], F32, name="rcs", tag="rcs")
                        with nc.allow_low_precision(reason="f32r rounding for matmul rhs"):
                            nc.vector.reciprocal(_r(rcs), sps[j][0:1, :])
                        bps = tri_ps.tile([P, 512], F32, name="bps", tag="tri")
                        nc.tensor.matmul(bps, _r(ones_row), _r(rcs),
                                         start=True, stop=True)
                        bcs = bcs_pool.tile([P, 512], F32)
                        nc.scalar.copy(bcs, bps)
                        for et in range(CT):
                            sl = out2[et][:, j * 512:(j + 1) * 512]
                            nc.vector.tensor_mul(sl, o2ps[j][et], bcs)
                            dmp = dump_pool.tile([P, 512], F32)
                            nc.scalar.activation(dmp, sl, AF.Square,
                                                 accum_out=q2_8[et][:, j:j + 1])
                            nc.vector.reduce_sum(s2_8[et][:, j:j + 1], sl,
                                                 axis=AX.X)
                        del o2ps[j], sps[j]

                prev2 = None
                for j in range(NCH):
                    o2ps[j] = [quad_ps.tile([P, 512], F32, name="o2ps", tag="quad") for _ in range(CT)]
                    sps[j] = row_ps.tile([1, 512], F32, name="sps", tag="row")
                    for dt in range(CT):
                        k2 = tri_ps.tile([P, 512], F32, name="k2", tag="tri")
                        for ct in range(CT):
                            nc.tensor.matmul(
                                k2,
                                _r(kvw_sb[:, ct * 2 * C + dt * P:
                                          ct * 2 * C + (dt + 1) * P]),
                                _r(xn[ct][:, j * 512:(j + 1) * 512]),
                                start=(ct == 0), stop=(ct == CT - 1))
                        ek2 = ek2_pool.tile([P, 512], F32)
                        nc.scalar.activation(_r(ek2), k2, AF.Exp,
                                             bias=kbc[:, dt:dt + 1])
                        if prev2 is not None:
                            emit_attn(*prev2)
                        prev2 = (j, dt, ek2)
                emit_attn(*prev2)

                # ================= GroupNorm 2 params =======================
                ab2 = []
                for et in range(CT):
                    st2 = stat_pool.tile([P, 2], F32)
                    nc.vector.reduce_sum(st2[:, 0:1], s2_8[et], axis=AX.X)
                    nc.vector.reduce_sum(st2[:, 1:2], q2_8[et], axis=AX.X)
                    ab2.append(gn_params(st2, w2c[:, et:et + 1], b2c[:, et:et + 1]))

                # ================= Phase 3: gelu + proj + residual ==========
                def emit_proj(j, gts):
                    for ot in range(CT):
                        o3 = quad_ps.tile([P, 512], F32, name="o3", tag="quad")
                        for et in range(CT):
                            nc.tensor.matmul(
                                o3,
                                _r(outw_sb[:, et * C + ot * P: et * C + (ot + 1) * P]),
                                _r(gts[et]),
                                start=(et == 0), stop=(et == CT - 1))
                        ob_sb = outsb_pool.tile([P, 512], F32)
                        nc.scalar.activation(ob_sb, o3, AF.Identity,
                                             bias=obc[:, ot:ot + 1])
                        nc.vector.tensor_add(ob_sb, ob_sb,
                                             _r(xn[ot][:, j * 512:(j + 1) * 512]))
                        nc.sync.dma_start(
                            out_d[row0 + ot * P: row0 + (ot + 1) * P,
                                  j * 512:(j + 1) * 512], ob_sb)

                prev3 = None
                for j in range(NCH):
                    gts = []
                    for et in range(CT):
                        g = g_pool.tile([P, 512], F32, name="g", tag="g")
                        nc.scalar.activation(_r(g), out2[et][:, j * 512:(j + 1) * 512],
                                             gelu_f, bias=ab2[et][:, 1:2],
                                             scale=ab2[et][:, 0:1])
                        gts.append(g)
                    if prev3 is not None:
                        emit_proj(*prev3)
                    prev3 = (j, gts)
                emit_proj(*prev3)

    nc.compile()
    return nc


def prep_inputs(inputs):
    """Host-side prep: shard x over batch, pre-transpose/pack weights."""
    x = np.ascontiguousarray(np.asarray(inputs["x"], dtype=np.float32))
    kv_w = np.asarray(inputs["kv_w"], dtype=np.float32)
    kv_b = np.asarray(inputs["kv_b"], dtype=np.float32)
    out_w = np.asarray(inputs["out_w"], dtype=np.float32)
    out_b = np.asarray(inputs["out_b"], dtype=np.float32)
    w1 = np.asarray(inputs["norm1_w"], dtype=np.float32)
    b1 = np.asarray(inputs["norm1_b"], dtype=np.float32)
    w2 = np.asarray(inputs["norm2_w"], dtype=np.float32)
    b2 = np.asarray(inputs["norm2_b"], dtype=np.float32)

    kvwT = np.ascontiguousarray(kv_w.T)                 # [C, 2C]
    outwT = np.ascontiguousarray(out_w.T)               # [C, C]
    kb = kv_b[:C]
    vb = np.ascontiguousarray(kv_b[C:]).reshape(1, C)
    prm = np.stack([w1, b1, kb, w2, b2, out_b]).reshape(6, CT, P)
    prm = np.ascontiguousarray(prm)
    gmat = np.zeros((P, 4), np.float32)
    for p in range(P):
        gmat[p, p // GSIZE] = 1.0
    gmatT = np.ascontiguousarray(gmat.T)

    xs = x.reshape(B, C, N)
    in_maps = []
    for i in range(N_CORES):
        shard = np.ascontiguousarray(
            xs[i * BPC:(i + 1) * BPC].reshape(BPC * C, N))
        in_maps.append({
            "x": shard, "kvwT": kvwT, "outwT": outwT, "prm": prm,
            "vb": vb, "gmat": gmat, "gmatT": gmatT,
        })
    return in_maps


_NC_CACHE = {}


def get_program(gelu: bool = True, reps: int = 1):
    key = (bool(gelu), reps)
    if key not in _NC_CACHE:
        _NC_CACHE[key] = build_program(gelu=key[0], reps=reps)
    return _NC_CACHE[key]


def run(inputs, trace: bool = False, gelu: bool = True, reps: int = 1):
    """Run on 8 cores; returns (full_output [16,512,64,64], BassKernelResults)."""
    nc = get_program(gelu=gelu, reps=reps)
    in_maps = prep_inputs(inputs)
    res = run_bass_kernel_spmd(nc, in_maps, core_ids=list(range(N_CORES)),
                               trace=trace)
    full = np.empty((B, C, N), np.float32)
    for i in range(N_CORES):
        full[i * BPC:(i + 1) * BPC] = res.results[i]["out"].reshape(BPC, C, N)
    return full.reshape(B, C, H, W), res


def kernel(**inputs) -> np.ndarray:
    out, _ = run(inputs, trace=False, gelu=True)
    return out
